# revision 13
# baseline (speedup 1.0000x reference)
"""BiCutLoss Trainium2 kernel (8-core data parallel over batch).

Reference semantics (B=16384, L=1024):
    temp[b,j]  = argmax(output[b,j,:])          # 1 iff out1 > out0 (ties -> 0)
    idx[b]     = L if row all-ones else index of last zero
    mask[b,j]  = j <= idx[b]
    r1[b,j]    = -1/log2(j+2)  if labels==1 else (j+1)/alpha
    loss       = sum(output[...,1] * mask * r1) / B

Key restructuring: masked_sum = full_sum - tail_sum, where the tail
(j > idx) is confined to the last W columns whenever each row has a zero
decision in its last W positions. For +-symmetric random data
P(no zero in last W=128) = 2^-128 per row; a per-row flag detects the
(cosmically unlikely / adversarial-only) violation and the host falls
back to an exact numpy evaluation, so the kernel is correct for all
inputs. Benefits: out0 is only read on the window (1/8 of it), and the
compare/scan/mask work runs on [128, W] tiles instead of [128, L].

Full sums, two routes balancing VectorE vs TensorE:
  PE route  (most tiles): ql = out1*lab on DVE; colsum(out1), colsum(ql)
             via ones^T-matmul into PSUM; epilogue dots with Bv / D.
  DVE route (a few tiles): r1 = lab*D + Bv materialized against
             partition-broadcast D/Bv tiles; fused (r1*out1) multiply +
             row-sum accumulation in one scalar_tensor_tensor.
Tail sums mirror the same two routes on the window slice.
Host sums the per-core partials and divides by B.
"""

import threading
from contextlib import ExitStack

import numpy as np

B, L = 16384, 1024
N_CORES = 8
ROWS_PER_CORE = B // N_CORES  # 2048
ALPHA = 0.65
W = 128  # tail window width
DVE_ROUTE_TILES = 6  # tiles whose full-sum runs entirely on VectorE

_compiled = threading.local()


def _reward_rows():
    j = np.arange(L, dtype=np.float64)
    bv = (j + 1.0) / ALPHA
    d = -1.0 / np.log2(j + 2.0) - bv
    return bv.astype(np.float32), d.astype(np.float32)


def _build(rows=ROWS_PER_CORE, num_devices=N_CORES, dve_route_tiles=DVE_ROUTE_TILES):
    import concourse.tile as tile
    from concourse import bacc, mybir

    f32 = mybir.dt.float32
    f16 = mybir.dt.float16
    u8 = mybir.dt.uint8
    Alu = mybir.AluOpType
    Act = mybir.ActivationFunctionType

    n_tiles = rows // 128
    n_dve = min(dve_route_tiles, n_tiles)
    n_pe = n_tiles - n_dve

    nc = bacc.Bacc(
        "TRN2",
        target_bir_lowering=False,
        debug=False,
        enable_asserts=True,
        num_devices=num_devices,
    )

    out1_d = nc.dram_tensor("out1", [rows, L], f32, kind="ExternalInput").ap()
    out0w_d = nc.dram_tensor("out0w", [rows, W], f32, kind="ExternalInput").ap()
    lab_d = nc.dram_tensor("lab", [rows, L], u8, kind="ExternalInput").ap()
    bv_d = nc.dram_tensor("bv", [1, L], f32, kind="ExternalInput").ap()
    dd_d = nc.dram_tensor("dd", [1, L], f32, kind="ExternalInput").ap()
    # partition-broadcast copies for the DVE route ([128, L], same row repeated)
    bvb_d = nc.dram_tensor("bvb", [128, L], f32, kind="ExternalInput").ap()
    ddb_d = nc.dram_tensor("ddb", [128, L], f32, kind="ExternalInput").ap()
    res_d = nc.dram_tensor("partial", [1, 8], f32, kind="ExternalOutput").ap()
    flag_d = nc.dram_tensor("flags", [128, n_tiles], f32, kind="ExternalOutput").ap()

    with tile.TileContext(nc) as tc, ExitStack() as ctx:
        const = ctx.enter_context(tc.tile_pool(name="const", bufs=1))
        inp = ctx.enter_context(tc.tile_pool(name="inp", bufs=6))
        work = ctx.enter_context(tc.tile_pool(name="work", bufs=6))
        small = ctx.enter_context(tc.tile_pool(name="small", bufs=4))
        psum = ctx.enter_context(tc.tile_pool(name="psum", bufs=1, space="PSUM"))

        ones = const.tile([128, 1], f32)
        nc.vector.memset(ones[:], 1.0)
        bv_row = const.tile([1, L], f32)
        nc.scalar.dma_start(bv_row[:], bv_d[:])
        d_row = const.tile([1, L], f32)
        nc.scalar.dma_start(d_row[:], dd_d[:])
        bvb = const.tile([128, L], f32)
        nc.scalar.dma_start(bvb[:], bvb_d[:])
        ddb = const.tile([128, L], f32)
        nc.scalar.dma_start(ddb[:], ddb_d[:])

        flag_t = const.tile([128, n_tiles], f32)

        # PSUM accumulators: full colsums (PE route) + window tail colsums
        psq_a = psum.tile([1, 512], f32)
        psq_b = psum.tile([1, 512], f32)
        psl_a = psum.tile([1, 512], f32)
        psl_b = psum.tile([1, 512], f32)
        psw_q = psum.tile([1, W], f32)
        psw_l = psum.tile([1, W], f32)

        # DVE-route accumulators
        acc_main = const.tile([128, 1], f32)
        nc.vector.memset(acc_main[:], 0.0)
        acc_tail = const.tile([128, 1], f32)
        nc.vector.memset(acc_tail[:], 0.0)

        stride = max(1, n_tiles // max(n_dve, 1))
        dve_set = set((k * stride + stride - 1) % n_tiles for k in range(n_dve))
        n_pe_seen = 0
        for i in range(n_tiles):
            r0 = i * 128
            dve_route = i in dve_set
            out1_t = inp.tile([128, L], f32)
            nc.sync.dma_start(out1_t[:, L - W : L], out1_d[r0 : r0 + 128, L - W : L])
            out0w_t = inp.tile([128, W], f32)
            nc.sync.dma_start(out0w_t[:], out0w_d[r0 : r0 + 128, :])
            nc.gpsimd.dma_start(out1_t[:, 0 : L - W], out1_d[r0 : r0 + 128, 0 : L - W])
            lab_t = inp.tile([128, L], u8)
            nc.gpsimd.dma_start(lab_t[:], lab_d[r0 : r0 + 128, :])

            out1_w = out1_t[:, L - W : L]

            # ---- window mask: ge -> suffix-max s -> tail mask tm ----
            ge_w = work.tile([128, W], f16, tag="gew")
            nc.vector.tensor_tensor(ge_w[:], out0w_t[:], out1_w, Alu.is_ge)
            s_w = work.tile([128, W], f16, tag="sw")
            nc.vector.tensor_tensor_scan(
                s_w[:, ::-1], ge_w[:, ::-1], ge_w[:, ::-1], 0.0, Alu.max, Alu.max
            )
            # ao = 1 iff no zero decision inside the window (suspicious OR
            # genuinely all-ones row; either way tail contribution -> 0 and
            # the flag lets the host decide).
            nc.vector.tensor_scalar(
                flag_t[:, i : i + 1], s_w[:, 0:1], 0.0, None, Alu.is_equal
            )
            omao_col = small.tile([128, 1], f32, tag="omao")
            nc.vector.tensor_scalar(
                omao_col[:], flag_t[:, i : i + 1], -1.0, 1.0, Alu.mult, Alu.add
            )
            # tm = 1 - s - ao  (1 on the strict tail j > idx, else 0) on ScalarE
            tm_w = work.tile([128, W], f32, tag="tmw")
            nc.scalar.activation(
                tm_w[:], s_w[:], Act.Identity, bias=omao_col[:], scale=-1.0
            )

            if dve_route:
                # r1 = lab*D + Bv ; main = sum_j r1*out1 ; w kept for tail
                t1 = work.tile([128, L], f32, tag="t1")
                nc.vector.tensor_tensor(t1[:], lab_t[:], ddb[:], Alu.mult)
                r1 = work.tile([128, L], f32, tag="r1")
                nc.vector.tensor_tensor(r1[:], t1[:], bvb[:], Alu.add)
                wfull = work.tile([128, L], f32, tag="wfull")
                row_col = small.tile([128, 1], f32, tag="rowc")
                nc.vector.scalar_tensor_tensor(
                    wfull[:], r1[:], 1.0, out1_t[:], Alu.mult, Alu.mult,
                    accum_out=row_col[:],
                )
                nc.vector.tensor_tensor(acc_main[:], acc_main[:], row_col[:], Alu.add)
                # tail = sum_jw tm * w_window
                tail_col = small.tile([128, 1], f32, tag="tailc")
                junkw = work.tile([128, W], f32, tag="junkw")
                nc.vector.scalar_tensor_tensor(
                    junkw[:], tm_w[:], 1.0, wfull[:, L - W : L], Alu.mult, Alu.mult,
                    accum_out=tail_col[:],
                )
                nc.vector.tensor_tensor(acc_tail[:], acc_tail[:], tail_col[:], Alu.add)
            else:
                st, sp = n_pe_seen == 0, n_pe_seen == n_pe - 1
                n_pe_seen += 1
                # ql = out1 * lab
                ql = work.tile([128, L], f32, tag="ql")
                nc.vector.tensor_tensor(ql[:], out1_t[:], lab_t[:], Alu.mult)
                nc.tensor.matmul(psq_a[:], ones[:], out1_t[:, 0:512], start=st, stop=sp)
                nc.tensor.matmul(psq_b[:], ones[:], out1_t[:, 512:L], start=st, stop=sp)
                nc.tensor.matmul(psl_a[:], ones[:], ql[:, 0:512], start=st, stop=sp)
                nc.tensor.matmul(psl_b[:], ones[:], ql[:, 512:L], start=st, stop=sp)
                # tails: tail_q = tm*out1_w ; tail_ql = tail_q*lab_w
                tq = work.tile([128, W], f32, tag="tq")
                nc.vector.tensor_tensor(tq[:], tm_w[:], out1_w, Alu.mult)
                tl = work.tile([128, W], f32, tag="tl")
                nc.vector.tensor_tensor(tl[:], tq[:], lab_t[:, L - W : L], Alu.mult)
                nc.tensor.matmul(psw_q[:], ones[:], tq[:], start=st, stop=sp)
                nc.tensor.matmul(psw_l[:], ones[:], tl[:], start=st, stop=sp)

        # ---- DVE-route accumulator partition-sums via PE ----
        ps_m = psum.tile([1, 1], f32)
        ps_t = psum.tile([1, 1], f32)
        nc.tensor.matmul(ps_m[:], ones[:], acc_main[:], start=True, stop=True)
        nc.tensor.matmul(ps_t[:], ones[:], acc_tail[:], start=True, stop=True)

        # ---- epilogue: weighted dots straight out of PSUM ----
        def dot(ps_ap, row_ap, tag):
            junk = const.tile([1, ps_ap.shape[1]], f32, tag="junk" + tag)
            acc = const.tile([1, 1], f32, tag="acc" + tag)
            nc.vector.scalar_tensor_tensor(
                junk[:], ps_ap, 1.0, row_ap, Alu.mult, Alu.mult, accum_out=acc[:]
            )
            return acc

        s1a = dot(psq_a[:], bv_row[:, 0:512], "1a")
        s1b = dot(psq_b[:], bv_row[:, 512:L], "1b")
        s2a = dot(psl_a[:], d_row[:, 0:512], "2a")
        s2b = dot(psl_b[:], d_row[:, 512:L], "2b")
        s3 = dot(psw_q[:], bv_row[:, L - W : L], "3")
        s4 = dot(psw_l[:], d_row[:, L - W : L], "4")
        s5 = const.tile([1, 1], f32)
        nc.scalar.copy(s5[:], ps_m[:])
        s6 = const.tile([1, 1], f32)
        nc.scalar.copy(s6[:], ps_t[:])

        for k, t_ in enumerate([s1a, s1b, s2a, s2b, s3, s4, s5, s6]):
            nc.sync.dma_start(res_d[0:1, k : k + 1], t_[:])
        nc.sync.dma_start(flag_d[:], flag_t[:])

    nc.compile()
    return nc


def _get_nc():
    if getattr(_compiled, "nc", None) is None:
        _compiled.nc = _build()
    return _compiled.nc


def _in_maps(output, labels):
    out1 = np.ascontiguousarray(output[:, :, 1], dtype=np.float32)
    out0w = np.ascontiguousarray(output[:, L - W : L, 0], dtype=np.float32)
    lab = labels.astype(np.uint8)  # values are 0/1
    bv, dd = _reward_rows()
    bvb = np.broadcast_to(bv, (128, L)).copy()
    ddb = np.broadcast_to(dd, (128, L)).copy()
    rp = ROWS_PER_CORE
    return [
        {
            "out1": out1[c * rp : (c + 1) * rp],
            "out0w": out0w[c * rp : (c + 1) * rp],
            "lab": lab[c * rp : (c + 1) * rp],
            "bv": bv.reshape(1, L),
            "dd": dd.reshape(1, L),
            "bvb": bvb,
            "ddb": ddb,
        }
        for c in range(N_CORES)
    ]


def _host_fallback(output, labels):
    temp = output[:, :, 1] > output[:, :, 0]
    allones = temp.all(axis=1)
    z = ~temp
    last_zero = (L - 1) - np.argmax(z[:, ::-1], axis=1)
    idx = np.where(allones, L, last_zero)
    mask = np.arange(L)[None, :] <= idx[:, None]
    j = np.arange(L, dtype=np.float64)
    r1 = np.where(labels == 1, -1.0 / np.log2(j + 2.0), (j + 1.0) / ALPHA)
    return np.float32(
        (output[:, :, 1].astype(np.float64) * mask * r1).sum() / B
    )


def _combine(results, output, labels):
    total = 0.0
    suspicious = 0.0
    for c, r in enumerate(results):
        p = np.asarray(r["partial"], dtype=np.float64)[0]
        total += p[0] + p[1] + p[2] + p[3] - p[4] - p[5] + p[6] - p[7]
        # rows flagged "no zero in window": genuine all-ones rows are handled
        # (tail = 0) but a row whose last zero is before the window is not —
        # recheck on host. Never fires for +-symmetric random inputs.
        flags = np.asarray(r["flags"], dtype=np.float64)
        if flags.max() > 0:
            rp = ROWS_PER_CORE
            o = output[c * rp : (c + 1) * rp]
            allones_rows = (o[:, :, 1] > o[:, :, 0]).all(axis=1)
            flagged = flags.T.reshape(-1) > 0  # row-major within this core
            suspicious += (flagged & ~allones_rows).sum()
    if suspicious > 0:
        return _host_fallback(output, labels)
    return np.float32(total / B)


def kernel(output: np.ndarray, labels: np.ndarray) -> np.ndarray:
    from concourse.bass_utils import run_bass_kernel_spmd

    assert output.shape == (B, L, 2), output.shape
    nc = _get_nc()
    res = run_bass_kernel_spmd(
        nc, _in_maps(output, labels), core_ids=list(range(N_CORES))
    )
    return _combine(res.results, output, labels)
